# revision 20
# baseline (speedup 1.0000x reference)
"""Trainium2 Bass kernel: 5x5 grayscale dilation (flat all-ones SE) =
5x5 stride-1 max-pool with geodesic (-1e4) border, over [8,3,2048,2048] f32.

Strategy (pure data-parallel over batch, 1 image per NeuronCore):
- "Patch" layout: each SBUF partition holds one horizontal band of
  hsub(+4 halo) image rows x Wt columns, so BOTH the H- and W-direction
  window maxes are free-dimension shifts (no cross-partition ops).
- Separable max: 3 pairwise-max ops per direction (window 5 = cascade
  2/4/5) on the DVE, with buffer reuse and an in-place final max so
  12 large tiles (hsub=64, strips=4) fit in SBUF (fewer, bigger ops
  amortize per-instruction overhead; this walrus build rejects GPSIMD
  tensor ops, so compute is DVE-only).
- DMA via HWDGE (nc.sync for loads, nc.scalar for stores) so descriptor
  generation never touches GPSIMD and loads/stores sit on different
  hardware rings.
"""

import sys

import numpy as np

for _p in ("/opt/trn_rl_repo",):
    if _p not in sys.path:
        sys.path.insert(0, _p)

NEG = -10000.0  # matches reference MAX_VAL padding


def build_pool_nc(C, H, W, hsub=64, wt_valid=128, strips=4, dve_rows_w=99,
                  dve_rows_h=99, variant="plain2", dve_frac=1.0, reps=1):
    """Build the single-core Bass program for a [C,H,W] f32 5x5 max pool."""
    from contextlib import ExitStack

    import concourse.bass as bass  # noqa: F401
    import concourse.mybir as mybir
    import concourse.tile as tile
    from concourse import bacc
    from bass_rust import AP

    f32 = mybir.dt.float32
    bands = H // hsub
    assert bands * hsub == H
    P = strips * bands
    assert P <= 128
    tile_w = strips * wt_valid
    n_wt = W // tile_w
    assert n_wt * tile_w == W
    Wt = wt_valid + 4  # loaded cols per strip (2 halo each side)
    hh = hsub + 4      # loaded rows per band
    wv = wt_valid
    Hp, Wp = H + 4, W + 4  # host-padded input (NEG border)
    ppitch = hh * Wt       # in-tile per-partition elements
    opitch = hsub * wv     # out-tile per-partition elements

    nc = bacc.Bacc()
    img = nc.declare_dram_parameter("image", [C, Hp, Wp], f32,
                                    isOutput=False)
    outp = nc.declare_dram_parameter("out", [C, H, W], f32, isOutput=True)

    ha = min(dve_rows_w, hh)   # DVE W-pass rows [0, ha)
    hb = min(dve_rows_h, hsub)  # DVE H-pass output rows [0, hb)

    with tile.TileContext(nc) as tc, ExitStack() as ctx:
        pin = ctx.enter_context(tc.tile_pool(name="pin", bufs=2))
        pu = ctx.enter_context(tc.tile_pool(name="pu", bufs=1))
        pv = ctx.enter_context(tc.tile_pool(name="pv", bufs=1))
        pR = ctx.enter_context(tc.tile_pool(
            name="pR", bufs=1 if variant == "dec2" else 2))
        ps = ctx.enter_context(tc.tile_pool(name="ps", bufs=1))
        pt = ctx.enter_context(tc.tile_pool(name="pt", bufs=1))
        pout = ctx.enter_context(tc.tile_pool(name="pout", bufs=2))

        for rep in range(reps):
          for ch in range(C):
            for wi in range(n_wt):
                in_t = pin.tile([P, hh, Wt], f32)
                base = in_t[:]
                # one rectangular load per strip (input is host-padded)
                for s in range(strips):
                    col = wi * tile_w + s * wt_valid
                    sap = [[hsub * Wp, bands], [Wp, hh], [1, Wt]]
                    dap = [[ppitch, bands], [Wt, hh], [1, Wt]]
                    srcap = AP(img, ch * Hp * Wp + col, sap)
                    dst = AP(base.tensor,
                             base.offset + s * bands * ppitch, dap)
                    eng = nc.sync if s % 2 == 0 else nc.scalar
                    eng.dma_start(out=dst, in_=srcap)

                # ---- compute
                if variant == "copy":
                    # pure-DMA roofline probe: store loaded data back
                    ib = in_t[:]
                    for s in range(strips):
                        src_ = AP(ib.tensor,
                                  ib.offset + s * bands * ppitch + 2 * Wt + 2,
                                  [[ppitch, bands], [Wt, hsub], [1, wv]])
                        dst = AP(outp,
                                 ch * H * W + wi * tile_w + s * wt_valid,
                                 [[hsub * W, bands], [W, hsub], [1, wv]])
                        eng = nc.scalar if s % 2 == 0 else nc.sync
                        eng.dma_start(out=dst, in_=src_)
                    continue
                out_t = pout.tile([P, hsub, wv], f32)
                if variant == "plain2":
                    u = pu.tile([P, hh, Wt - 1], f32, tag="A")
                    v = pv.tile([P, hh, Wt - 3], f32, tag="B")
                    nc.vector.tensor_max(u[:], in_t[:, :, 0:Wt - 1],
                                         in_t[:, :, 1:Wt])
                    nc.vector.tensor_max(v[:], u[:, :, 0:Wt - 3],
                                         u[:, :, 2:Wt - 1])
                    R2 = pu.tile([P, hh, wv], f32, tag="A")
                    nc.vector.tensor_max(R2[:], v[:, :, 0:wv],
                                         in_t[:, :, 4:Wt])
                    s2 = pv.tile([P, hsub + 2, wv], f32, tag="B")
                    nc.vector.tensor_max(s2[:], R2[:, 0:hsub + 2, :],
                                         R2[:, 1:hsub + 3, :])
                    nc.vector.tensor_max(out_t[:], s2[:, 0:hsub, :],
                                         s2[:, 2:hsub + 2, :])
                    nc.vector.tensor_max(out_t[:], out_t[:],
                                         R2[:, 4:hsub + 4, :])
                    ob = out_t[:]
                    for s in range(strips):
                        src_ = AP(ob.tensor, ob.offset + s * bands * opitch,
                                  [[opitch, bands], [wv, hsub], [1, wv]])
                        dst = AP(outp,
                                 ch * H * W + wi * tile_w + s * wt_valid,
                                 [[hsub * W, bands], [W, hsub], [1, wv]])
                        eng = nc.scalar if s % 2 == 0 else nc.sync
                        eng.dma_start(out=dst, in_=src_)
                    continue
                if variant == "dec2":
                    # decimated pair/merge pyramid, DVE-only, tag-reuse
                    nh, nm = Wt // 2, wv // 2
                    nj, no = hh // 2, hsub // 2
                    p = pu.tile([P, hh, nh], f32, tag="A")
                    nc.vector.tensor_max(p[:], in_t[:, :, 0:2 * nh:2],
                                         in_t[:, :, 1:2 * nh:2])
                    t1 = pv.tile([P, hh, nm], f32, tag="B")
                    nc.vector.tensor_max(t1[:], p[:, :, 0:nm],
                                         p[:, :, 1:nm + 1])
                    R2 = pR.tile([P, hh, wv], f32)
                    nc.vector.tensor_max(R2[:, :, 0:wv:2], t1[:],
                                         in_t[:, :, 4:4 + 2 * nm:2])
                    t2 = pv.tile([P, hh, nm], f32, tag="B")
                    nc.vector.tensor_max(t2[:], p[:, :, 1:nm + 1],
                                         p[:, :, 2:nm + 2])
                    nc.vector.tensor_max(R2[:, :, 1:wv:2], t2[:],
                                         in_t[:, :, 1:1 + 2 * nm:2])
                    q = pu.tile([P, nj, wv], f32, tag="A")
                    nc.vector.tensor_max(q[:], R2[:, 0:2 * nj:2, :],
                                         R2[:, 1:2 * nj:2, :])
                    u1 = pv.tile([P, no, wv], f32, tag="B")
                    nc.vector.tensor_max(u1[:], q[:, 0:no, :],
                                         q[:, 1:no + 1, :])
                    nc.vector.tensor_max(out_t[:, 0:hsub:2, :], u1[:],
                                         R2[:, 4:4 + 2 * no:2, :])
                    u2 = pv.tile([P, no, wv], f32, tag="B")
                    nc.vector.tensor_max(u2[:], q[:, 1:no + 1, :],
                                         q[:, 2:no + 2, :])
                    nc.vector.tensor_max(out_t[:, 1:2 * no:2, :], u2[:],
                                         R2[:, 1:1 + 2 * no:2, :])
                    ob = out_t[:]
                    for s in range(strips):
                        src_ = AP(ob.tensor, ob.offset + s * bands * opitch,
                                  [[opitch, bands], [wv, hsub], [1, wv]])
                        dst = AP(outp,
                                 ch * H * W + wi * tile_w + s * wt_valid,
                                 [[hsub * W, bands], [W, hsub], [1, wv]])
                        eng = nc.scalar if s % 2 == 0 else nc.sync
                        eng.dma_start(out=dst, in_=src_)
                    continue
                R = pR.tile([P, hh, wv], f32)
                if variant == "plain":
                    u = pu.tile([P, hh, Wt - 1], f32)
                    v = pv.tile([P, hh, Wt - 3], f32)
                    st = ps.tile([P, hsub + 2, wv], f32)
                    tt = pt.tile([P, hsub, wv], f32)

                    # W-pass, rows split DVE [0,ha) / GPSIMD [ha,hh)
                    for eng, r0, r1 in ((nc.vector, 0, ha),
                                        (nc.gpsimd, ha, hh)):
                        if r0 >= r1:
                            continue
                        eng.tensor_max(u[:, r0:r1, :],
                                       in_t[:, r0:r1, 0:Wt - 1],
                                       in_t[:, r0:r1, 1:Wt])
                        eng.tensor_max(v[:, r0:r1, :],
                                       u[:, r0:r1, 0:Wt - 3],
                                       u[:, r0:r1, 2:Wt - 1])
                        eng.tensor_max(R[:, r0:r1, :],
                                       v[:, r0:r1, 0:wv],
                                       in_t[:, r0:r1, 4:Wt])

                    # H-pass, out rows split DVE [0,hb) / GPSIMD [hb,hsub)
                    for eng, q0, q1 in ((nc.vector, 0, hb),
                                        (nc.gpsimd, hb, hsub)):
                        if q0 >= q1:
                            continue
                        eng.tensor_max(st[:, q0:q1 + 2, :],
                                       R[:, q0:q1 + 2, :],
                                       R[:, q0 + 1:q1 + 3, :])
                        eng.tensor_max(tt[:, q0:q1, :],
                                       st[:, q0:q1, :],
                                       st[:, q0 + 2:q1 + 2, :])
                        eng.tensor_max(out_t[:, q0:q1, :],
                                       tt[:, q0:q1, :],
                                       R[:, q0 + 4:q1 + 4, :])
                else:
                    # Decimated: pair-max p then merge, per direction.
                    # W: R[2m]  = max(p[m], p[m+1], in[2m+4])
                    #    R[2m+1]= max(p[m+1], p[m+2], in[2m+1])
                    nh = Wt // 2           # pairs per row (66)
                    nm = wv // 2           # merge outputs per parity (64)
                    p = pu.tile([P, hh, nh], f32)
                    t1 = pv.tile([P, hh, nm], f32, tag="t1")
                    t2 = pv.tile([P, hh, nm], f32, tag="t2")
                    for eng, r0, r1 in ((nc.vector, 0, ha),
                                        (nc.gpsimd, ha, hh)):
                        if r0 >= r1:
                            continue
                        rr = slice(r0, r1)
                        eng.tensor_max(p[:, rr, :],
                                       in_t[:, rr, 0:2 * nh:2],
                                       in_t[:, rr, 1:2 * nh:2])
                        eng.tensor_max(t1[:, rr, :],
                                       p[:, rr, 0:nm],
                                       p[:, rr, 1:nm + 1])
                        eng.tensor_max(R[:, rr, 0:wv:2],
                                       t1[:, rr, :],
                                       in_t[:, rr, 4:4 + 2 * nm:2])
                        eng.tensor_max(t2[:, rr, :],
                                       p[:, rr, 1:nm + 1],
                                       p[:, rr, 2:nm + 2])
                        eng.tensor_max(R[:, rr, 1:wv:2],
                                       t2[:, rr, :],
                                       in_t[:, rr, 1:1 + 2 * nm:2])
                    # H: out[2j]  = max(q[j], q[j+1], R[2j+4])
                    #    out[2j+1]= max(q[j+1], q[j+2], R[2j+1])
                    nj = hh // 2           # 18
                    no = hsub // 2         # 16
                    q = ps.tile([P, nj, wv], f32)
                    u1 = pt.tile([P, no, wv], f32, tag="u1")
                    u2 = pt.tile([P, no, wv], f32, tag="u2")
                    jb = max(0, min(no, round(no * dve_frac)))
                    for eng, a0, a1 in ((nc.vector, 0, min(nj, jb + 2)),
                                        (nc.gpsimd, min(nj, jb + 2), nj)):
                        if a0 >= a1:
                            continue
                        eng.tensor_max(q[:, a0:a1, :],
                                       R[:, 2 * a0:2 * a1:2, :],
                                       R[:, 2 * a0 + 1:2 * a1:2, :])
                    for eng, j0, j1 in ((nc.vector, 0, jb),
                                        (nc.gpsimd, jb, no)):
                        if j0 >= j1:
                            continue
                        jj = slice(j0, j1)
                        eng.tensor_max(u1[:, jj, :],
                                       q[:, j0:j1, :],
                                       q[:, j0 + 1:j1 + 1, :])
                        eng.tensor_max(out_t[:, 2 * j0:2 * j1:2, :],
                                       u1[:, jj, :],
                                       R[:, 2 * j0 + 4:2 * j1 + 4:2, :])
                        eng.tensor_max(u2[:, jj, :],
                                       q[:, j0 + 1:j1 + 1, :],
                                       q[:, j0 + 2:j1 + 2, :])
                        eng.tensor_max(out_t[:, 2 * j0 + 1:2 * j1:2, :],
                                       u2[:, jj, :],
                                       R[:, 2 * j0 + 1:2 * j1:2, :])

                # ---- store, per strip, cross-balanced over the two rings
                ob = out_t[:]
                for s in range(strips):
                    src = AP(ob.tensor, ob.offset + s * bands * opitch,
                             [[opitch, bands], [wv, hsub], [1, wv]])
                    dst = AP(outp,
                             ch * H * W + wi * tile_w + s * wt_valid,
                             [[hsub * W, bands], [W, hsub], [1, wv]])
                    eng = nc.scalar if s % 2 == 0 else nc.sync
                    eng.dma_start(out=dst, in_=src)
    return nc


def _numpy_ref(image, se):
    """Slow exact fallback for a non-all-ones structuring element."""
    B, C, H, W = image.shape
    kh, kw = se.shape
    oy, ox = kh // 2, kw // 2
    pad = np.full((B, C, H + kh - 1, W + kw - 1), NEG, dtype=image.dtype)
    pad[:, :, oy:oy + H, ox:ox + W] = image
    neigh = np.where(se == 0, NEG, 0.0).astype(image.dtype)[::-1, ::-1]
    out = np.full((B, C, H, W), -np.inf, dtype=image.dtype)
    for i in range(kh):
        for j in range(kw):
            np.maximum(out, pad[:, :, i:i + H, j:j + W] + neigh[i, j], out)
    return out


def pad_host(image):
    """Pad [B?,C,H,W] with the reference's geodesic border value."""
    pw = [(0, 0)] * (image.ndim - 2) + [(2, 2), (2, 2)]
    return np.pad(image, pw, mode="constant", constant_values=NEG)


_CACHE = {}


def kernel(image, kernel):
    image = np.asarray(image, dtype=np.float32)
    se = np.asarray(kernel, dtype=np.float32)
    if se.shape != (5, 5) or np.any(se == 0):
        return _numpy_ref(image, se)

    B, C, H, W = image.shape
    from concourse.bass_utils import run_bass_kernel_spmd

    key = (C, H, W)
    if key not in _CACHE:
        nc0 = build_pool_nc(C, H, W)
        if not nc0.is_finalized():
            nc0.finalize()
        _CACHE[key] = nc0
    nc = _CACHE[key]

    n_cores = 8
    if B != n_cores or H % 128 or W % 512:
        return _numpy_ref(image, se)
    padded = pad_host(image)
    in_maps = [{"image": padded[i]} for i in range(B)]
    res = run_bass_kernel_spmd(nc, in_maps, list(range(n_cores)))
    out = np.stack([res.results[i]["out"] for i in range(B)], axis=0)
    return out


if __name__ == "__main__":
    import jax
    import jax.numpy as jnp

    key = jax.random.key(0)
    k1, _ = jax.random.split(key)
    image = np.asarray(jax.random.uniform(
        k1, (8, 3, 2048, 2048), dtype=jnp.float32))
    se = np.ones((5, 5), np.float32)
    out = kernel(image, se)
    ref = _numpy_ref(image, se)
    err = np.abs(out - ref).max()
    print("abs max err:", err)


# revision 21
# speedup vs baseline: 1.0975x; 1.0975x over previous
"""Trainium2 Bass kernel: 5x5 grayscale dilation (flat all-ones SE) =
5x5 stride-1 max-pool with geodesic (-1e4) border, over [8,3,2048,2048] f32.

Strategy (pure data-parallel over batch, 1 image per NeuronCore):
- "Patch" layout: each SBUF partition holds one horizontal band of
  hsub(+4 halo) image rows x Wt columns, so BOTH the H- and W-direction
  window maxes are free-dimension shifts (no cross-partition ops).
- Separable max: 3 pairwise-max ops per direction (window 5 = cascade
  2/4/5) on the DVE, with buffer reuse and an in-place final max so
  12 large tiles (hsub=64, strips=4) fit in SBUF (fewer, bigger ops
  amortize per-instruction overhead; this walrus build rejects GPSIMD
  tensor ops, so compute is DVE-only).
- DMA via HWDGE (nc.sync for loads, nc.scalar for stores) so descriptor
  generation never touches GPSIMD and loads/stores sit on different
  hardware rings.
"""

import sys

import numpy as np

for _p in ("/opt/trn_rl_repo",):
    if _p not in sys.path:
        sys.path.insert(0, _p)

NEG = -10000.0  # matches reference MAX_VAL padding


def build_pool_nc(C, H, W, hsub=64, wt_valid=128, strips=4, dve_rows_w=99,
                  dve_rows_h=99, variant="plain2", dve_frac=1.0, reps=1, wide_dma=0):
    """Build the single-core Bass program for a [C,H,W] f32 5x5 max pool."""
    from contextlib import ExitStack

    import concourse.bass as bass  # noqa: F401
    import concourse.mybir as mybir
    import concourse.tile as tile
    from concourse import bacc
    from bass_rust import AP

    f32 = mybir.dt.float32
    bands = H // hsub
    assert bands * hsub == H
    P = strips * bands
    assert P <= 128
    tile_w = strips * wt_valid
    n_wt = W // tile_w
    assert n_wt * tile_w == W
    Wt = wt_valid + 4  # loaded cols per strip (2 halo each side)
    hh = hsub + 4      # loaded rows per band
    wv = wt_valid
    Hp, Wp = H + 4, W + 4  # host-padded input (NEG border)
    ppitch = hh * Wt       # in-tile per-partition elements
    opitch = hsub * wv     # out-tile per-partition elements

    nc = bacc.Bacc()
    img = nc.declare_dram_parameter("image", [C, Hp, Wp], f32,
                                    isOutput=False)
    outp = nc.declare_dram_parameter("out", [C, H, W], f32, isOutput=True)

    ha = min(dve_rows_w, hh)   # DVE W-pass rows [0, ha)
    hb = min(dve_rows_h, hsub)  # DVE H-pass output rows [0, hb)

    with tile.TileContext(nc) as tc, ExitStack() as ctx:
        pin = ctx.enter_context(tc.tile_pool(name="pin", bufs=2))
        pu = ctx.enter_context(tc.tile_pool(name="pu", bufs=1))
        pv = ctx.enter_context(tc.tile_pool(name="pv", bufs=1))
        pR = ctx.enter_context(tc.tile_pool(
            name="pR", bufs=1 if variant == "dec2" else 2))
        ps = ctx.enter_context(tc.tile_pool(name="ps", bufs=1))
        pt = ctx.enter_context(tc.tile_pool(name="pt", bufs=1))
        pout = ctx.enter_context(tc.tile_pool(name="pout", bufs=2))

        for rep in range(reps):
          for ch in range(C):
            for wi in range(n_wt):
                in_t = pin.tile([P, hh, Wt], f32)
                base = in_t[:]
                if wide_dma:
                    # one DMA spanning all strips/128 partitions (16 ports);
                    # 2-level partition dst AP is HW-fine (only CoreSim's
                    # shadow tracking dislikes it).
                    col = wi * tile_w
                    sap = [[wt_valid, strips], [hsub * Wp, bands],
                           [Wp, hh], [1, Wt]]
                    dap = [[bands * ppitch, strips], [ppitch, bands],
                           [Wt, hh], [1, Wt]]
                    nc.sync.dma_start(
                        out=AP(base.tensor, base.offset, dap),
                        in_=AP(img, ch * Hp * Wp + col, sap))
                else:
                    # one rectangular load per strip (input is host-padded)
                    for s in range(strips):
                        col = wi * tile_w + s * wt_valid
                        sap = [[hsub * Wp, bands], [Wp, hh], [1, Wt]]
                        dap = [[ppitch, bands], [Wt, hh], [1, Wt]]
                        srcap = AP(img, ch * Hp * Wp + col, sap)
                        dst = AP(base.tensor,
                                 base.offset + s * bands * ppitch, dap)
                        eng = nc.sync if s % 2 == 0 else nc.scalar
                        eng.dma_start(out=dst, in_=srcap)

                # ---- compute
                if variant == "copy":
                    # pure-DMA roofline probe: store loaded data back
                    ib = in_t[:]
                    for s in range(strips):
                        src_ = AP(ib.tensor,
                                  ib.offset + s * bands * ppitch + 2 * Wt + 2,
                                  [[ppitch, bands], [Wt, hsub], [1, wv]])
                        dst = AP(outp,
                                 ch * H * W + wi * tile_w + s * wt_valid,
                                 [[hsub * W, bands], [W, hsub], [1, wv]])
                        eng = nc.scalar if s % 2 == 0 else nc.sync
                        eng.dma_start(out=dst, in_=src_)
                    continue
                out_t = pout.tile([P, hsub, wv], f32)
                if variant == "plain2":
                    u = pu.tile([P, hh, Wt - 1], f32, tag="A")
                    v = pv.tile([P, hh, Wt - 3], f32, tag="B")
                    nc.vector.tensor_max(u[:], in_t[:, :, 0:Wt - 1],
                                         in_t[:, :, 1:Wt])
                    nc.vector.tensor_max(v[:], u[:, :, 0:Wt - 3],
                                         u[:, :, 2:Wt - 1])
                    R2 = pu.tile([P, hh, wv], f32, tag="A")
                    nc.vector.tensor_max(R2[:], v[:, :, 0:wv],
                                         in_t[:, :, 4:Wt])
                    s2 = pv.tile([P, hsub + 2, wv], f32, tag="B")
                    nc.vector.tensor_max(s2[:], R2[:, 0:hsub + 2, :],
                                         R2[:, 1:hsub + 3, :])
                    nc.vector.tensor_max(out_t[:], s2[:, 0:hsub, :],
                                         s2[:, 2:hsub + 2, :])
                    nc.vector.tensor_max(out_t[:], out_t[:],
                                         R2[:, 4:hsub + 4, :])
                    ob = out_t[:]
                    for s in range(strips):
                        src_ = AP(ob.tensor, ob.offset + s * bands * opitch,
                                  [[opitch, bands], [wv, hsub], [1, wv]])
                        dst = AP(outp,
                                 ch * H * W + wi * tile_w + s * wt_valid,
                                 [[hsub * W, bands], [W, hsub], [1, wv]])
                        eng = nc.scalar if s % 2 == 0 else nc.sync
                        eng.dma_start(out=dst, in_=src_)
                    continue
                if variant == "dec2":
                    # decimated pair/merge pyramid, DVE-only, tag-reuse
                    nh, nm = Wt // 2, wv // 2
                    nj, no = hh // 2, hsub // 2
                    p = pu.tile([P, hh, nh], f32, tag="A")
                    nc.vector.tensor_max(p[:], in_t[:, :, 0:2 * nh:2],
                                         in_t[:, :, 1:2 * nh:2])
                    t1 = pv.tile([P, hh, nm], f32, tag="B")
                    nc.vector.tensor_max(t1[:], p[:, :, 0:nm],
                                         p[:, :, 1:nm + 1])
                    R2 = pR.tile([P, hh, wv], f32)
                    nc.vector.tensor_max(R2[:, :, 0:wv:2], t1[:],
                                         in_t[:, :, 4:4 + 2 * nm:2])
                    t2 = pv.tile([P, hh, nm], f32, tag="B")
                    nc.vector.tensor_max(t2[:], p[:, :, 1:nm + 1],
                                         p[:, :, 2:nm + 2])
                    nc.vector.tensor_max(R2[:, :, 1:wv:2], t2[:],
                                         in_t[:, :, 1:1 + 2 * nm:2])
                    q = pu.tile([P, nj, wv], f32, tag="A")
                    nc.vector.tensor_max(q[:], R2[:, 0:2 * nj:2, :],
                                         R2[:, 1:2 * nj:2, :])
                    u1 = pv.tile([P, no, wv], f32, tag="B")
                    nc.vector.tensor_max(u1[:], q[:, 0:no, :],
                                         q[:, 1:no + 1, :])
                    nc.vector.tensor_max(out_t[:, 0:hsub:2, :], u1[:],
                                         R2[:, 4:4 + 2 * no:2, :])
                    u2 = pv.tile([P, no, wv], f32, tag="B")
                    nc.vector.tensor_max(u2[:], q[:, 1:no + 1, :],
                                         q[:, 2:no + 2, :])
                    nc.vector.tensor_max(out_t[:, 1:2 * no:2, :], u2[:],
                                         R2[:, 1:1 + 2 * no:2, :])
                    ob = out_t[:]
                    for s in range(strips):
                        src_ = AP(ob.tensor, ob.offset + s * bands * opitch,
                                  [[opitch, bands], [wv, hsub], [1, wv]])
                        dst = AP(outp,
                                 ch * H * W + wi * tile_w + s * wt_valid,
                                 [[hsub * W, bands], [W, hsub], [1, wv]])
                        eng = nc.scalar if s % 2 == 0 else nc.sync
                        eng.dma_start(out=dst, in_=src_)
                    continue
                R = pR.tile([P, hh, wv], f32)
                if variant == "plain":
                    u = pu.tile([P, hh, Wt - 1], f32)
                    v = pv.tile([P, hh, Wt - 3], f32)
                    st = ps.tile([P, hsub + 2, wv], f32)
                    tt = pt.tile([P, hsub, wv], f32)

                    # W-pass, rows split DVE [0,ha) / GPSIMD [ha,hh)
                    for eng, r0, r1 in ((nc.vector, 0, ha),
                                        (nc.gpsimd, ha, hh)):
                        if r0 >= r1:
                            continue
                        eng.tensor_max(u[:, r0:r1, :],
                                       in_t[:, r0:r1, 0:Wt - 1],
                                       in_t[:, r0:r1, 1:Wt])
                        eng.tensor_max(v[:, r0:r1, :],
                                       u[:, r0:r1, 0:Wt - 3],
                                       u[:, r0:r1, 2:Wt - 1])
                        eng.tensor_max(R[:, r0:r1, :],
                                       v[:, r0:r1, 0:wv],
                                       in_t[:, r0:r1, 4:Wt])

                    # H-pass, out rows split DVE [0,hb) / GPSIMD [hb,hsub)
                    for eng, q0, q1 in ((nc.vector, 0, hb),
                                        (nc.gpsimd, hb, hsub)):
                        if q0 >= q1:
                            continue
                        eng.tensor_max(st[:, q0:q1 + 2, :],
                                       R[:, q0:q1 + 2, :],
                                       R[:, q0 + 1:q1 + 3, :])
                        eng.tensor_max(tt[:, q0:q1, :],
                                       st[:, q0:q1, :],
                                       st[:, q0 + 2:q1 + 2, :])
                        eng.tensor_max(out_t[:, q0:q1, :],
                                       tt[:, q0:q1, :],
                                       R[:, q0 + 4:q1 + 4, :])
                else:
                    # Decimated: pair-max p then merge, per direction.
                    # W: R[2m]  = max(p[m], p[m+1], in[2m+4])
                    #    R[2m+1]= max(p[m+1], p[m+2], in[2m+1])
                    nh = Wt // 2           # pairs per row (66)
                    nm = wv // 2           # merge outputs per parity (64)
                    p = pu.tile([P, hh, nh], f32)
                    t1 = pv.tile([P, hh, nm], f32, tag="t1")
                    t2 = pv.tile([P, hh, nm], f32, tag="t2")
                    for eng, r0, r1 in ((nc.vector, 0, ha),
                                        (nc.gpsimd, ha, hh)):
                        if r0 >= r1:
                            continue
                        rr = slice(r0, r1)
                        eng.tensor_max(p[:, rr, :],
                                       in_t[:, rr, 0:2 * nh:2],
                                       in_t[:, rr, 1:2 * nh:2])
                        eng.tensor_max(t1[:, rr, :],
                                       p[:, rr, 0:nm],
                                       p[:, rr, 1:nm + 1])
                        eng.tensor_max(R[:, rr, 0:wv:2],
                                       t1[:, rr, :],
                                       in_t[:, rr, 4:4 + 2 * nm:2])
                        eng.tensor_max(t2[:, rr, :],
                                       p[:, rr, 1:nm + 1],
                                       p[:, rr, 2:nm + 2])
                        eng.tensor_max(R[:, rr, 1:wv:2],
                                       t2[:, rr, :],
                                       in_t[:, rr, 1:1 + 2 * nm:2])
                    # H: out[2j]  = max(q[j], q[j+1], R[2j+4])
                    #    out[2j+1]= max(q[j+1], q[j+2], R[2j+1])
                    nj = hh // 2           # 18
                    no = hsub // 2         # 16
                    q = ps.tile([P, nj, wv], f32)
                    u1 = pt.tile([P, no, wv], f32, tag="u1")
                    u2 = pt.tile([P, no, wv], f32, tag="u2")
                    jb = max(0, min(no, round(no * dve_frac)))
                    for eng, a0, a1 in ((nc.vector, 0, min(nj, jb + 2)),
                                        (nc.gpsimd, min(nj, jb + 2), nj)):
                        if a0 >= a1:
                            continue
                        eng.tensor_max(q[:, a0:a1, :],
                                       R[:, 2 * a0:2 * a1:2, :],
                                       R[:, 2 * a0 + 1:2 * a1:2, :])
                    for eng, j0, j1 in ((nc.vector, 0, jb),
                                        (nc.gpsimd, jb, no)):
                        if j0 >= j1:
                            continue
                        jj = slice(j0, j1)
                        eng.tensor_max(u1[:, jj, :],
                                       q[:, j0:j1, :],
                                       q[:, j0 + 1:j1 + 1, :])
                        eng.tensor_max(out_t[:, 2 * j0:2 * j1:2, :],
                                       u1[:, jj, :],
                                       R[:, 2 * j0 + 4:2 * j1 + 4:2, :])
                        eng.tensor_max(u2[:, jj, :],
                                       q[:, j0 + 1:j1 + 1, :],
                                       q[:, j0 + 2:j1 + 2, :])
                        eng.tensor_max(out_t[:, 2 * j0 + 1:2 * j1:2, :],
                                       u2[:, jj, :],
                                       R[:, 2 * j0 + 1:2 * j1:2, :])

                # ---- store, per strip, cross-balanced over the two rings
                ob = out_t[:]
                for s in range(strips):
                    src = AP(ob.tensor, ob.offset + s * bands * opitch,
                             [[opitch, bands], [wv, hsub], [1, wv]])
                    dst = AP(outp,
                             ch * H * W + wi * tile_w + s * wt_valid,
                             [[hsub * W, bands], [W, hsub], [1, wv]])
                    eng = nc.scalar if s % 2 == 0 else nc.sync
                    eng.dma_start(out=dst, in_=src)
    return nc


def _numpy_ref(image, se):
    """Slow exact fallback for a non-all-ones structuring element."""
    B, C, H, W = image.shape
    kh, kw = se.shape
    oy, ox = kh // 2, kw // 2
    pad = np.full((B, C, H + kh - 1, W + kw - 1), NEG, dtype=image.dtype)
    pad[:, :, oy:oy + H, ox:ox + W] = image
    neigh = np.where(se == 0, NEG, 0.0).astype(image.dtype)[::-1, ::-1]
    out = np.full((B, C, H, W), -np.inf, dtype=image.dtype)
    for i in range(kh):
        for j in range(kw):
            np.maximum(out, pad[:, :, i:i + H, j:j + W] + neigh[i, j], out)
    return out


def pad_host(image):
    """Pad [B?,C,H,W] with the reference's geodesic border value."""
    pw = [(0, 0)] * (image.ndim - 2) + [(2, 2), (2, 2)]
    return np.pad(image, pw, mode="constant", constant_values=NEG)


_CACHE = {}


def kernel(image, kernel):
    image = np.asarray(image, dtype=np.float32)
    se = np.asarray(kernel, dtype=np.float32)
    if se.shape != (5, 5) or np.any(se == 0):
        return _numpy_ref(image, se)

    B, C, H, W = image.shape
    from concourse.bass_utils import run_bass_kernel_spmd

    key = (C, H, W)
    if key not in _CACHE:
        nc0 = build_pool_nc(C, H, W)
        if not nc0.is_finalized():
            nc0.finalize()
        _CACHE[key] = nc0
    nc = _CACHE[key]

    n_cores = 8
    if B != n_cores or H % 128 or W % 512:
        return _numpy_ref(image, se)
    padded = pad_host(image)
    in_maps = [{"image": padded[i]} for i in range(B)]
    res = run_bass_kernel_spmd(nc, in_maps, list(range(n_cores)))
    out = np.stack([res.results[i]["out"] for i in range(B)], axis=0)
    return out


if __name__ == "__main__":
    import jax
    import jax.numpy as jnp

    key = jax.random.key(0)
    k1, _ = jax.random.split(key)
    image = np.asarray(jax.random.uniform(
        k1, (8, 3, 2048, 2048), dtype=jnp.float32))
    se = np.ones((5, 5), np.float32)
    out = kernel(image, se)
    ref = _numpy_ref(image, se)
    err = np.abs(out - ref).max()
    print("abs max err:", err)
